# revision 17
# baseline (speedup 1.0000x reference)
"""Trainium2 Bass kernel for nn_FactoredYiJingQuantizer.

Math: the 8 trigrams are all sign vectors {-1,+1}^3, so the softmax over
codebook entries factorizes per coordinate:
    w_k ∝ exp(-(|z|^2 - 2<z,s_k> + 3)/T) ∝ prod_d exp(2 z_d s_{k,d} / T)
    E[s_d] = tanh(2 z_d / T)
and the straight-through output x + sg(q - x) is numerically just q.
Hence the whole module is elementwise  y = tanh(x * 2/TEMP)  with
TEMP = 0.3 — a pure memory-bound elementwise kernel.

Sharding: data-parallel over the batch dim across 8 NeuronCores.

Perf: HBM traffic is the only roofline that matters (all 16 DMA engines
~80% busy in the f32 baseline trace), so the kernel trades precision for
bytes: inputs are cast host-side to fp8_e4m3 (bit-compatible with TRN
FP8_EXP4 for |x|<240) and outputs are stored as int8 = round(127*tanh),
dequantized on the host. Measured rel_l2 error ~5e-3 vs the 2e-2 gate.
Per-core traffic drops 4x vs f32/f32: 12.6MB in + 12.6MB out.

Engines: scalar (ACTIVATE Tanh, fp8->fp16, ~7.1us/tile), DVE
(tensor_scalar_mul x127, fp16->int8), DMA on the sync queue.
"""

import ml_dtypes
import numpy as np

import concourse.bacc as bacc
import concourse.mybir as mybir
from concourse.bass_utils import run_bass_kernel_spmd
from concourse.tile import TileContext

N_CORES = 8
B, S, D = 2048, 8192, 6
ROWS_PER_CORE = B // N_CORES                 # 256
ELEMS_PER_CORE = ROWS_PER_CORE * S * D       # 12,582,912
P = 128                                      # SBUF partitions
FREE_PER_CORE = ELEMS_PER_CORE // P          # 98,304 free elems per partition
# Small first tiles let the scalar engine start early (first-load DMA
# latency, not bandwidth, gates the ramp). The last 8192 is split into
# 2x4096 so the DVE x127 pass of the big tile overlaps the final TANHs,
# and tiny drain tiles shrink the final DVE+store serialization.
TILE_SIZES = [2048, 4096, 4096, 4096] + [8192] * 9 + [4096, 4096, 1024, 512, 512]
assert sum(TILE_SIZES) == FREE_PER_CORE
TEMP = 0.3
SCALE = 2.0 / TEMP
OUT_SCALE = 127.0

IN_NP_DT = ml_dtypes.float8_e4m3
OUT_NP_DT = np.int8
IN_MYBIR_DT = mybir.dt.float8e4
MID_MYBIR_DT = mybir.dt.float16
OUT_MYBIR_DT = mybir.dt.int8

_CACHE: dict = {}


def build_bass(
    tile_sizes: list[int] | None = None,
    in_bufs: int = 6,
    bufs: int = 4,
    enable_asserts: bool | None = None,
):
    tiles = list(tile_sizes or TILE_SIZES)
    assert sum(tiles) == FREE_PER_CORE
    tmax = max(tiles)
    nc = bacc.Bacc(num_devices=N_CORES, enable_asserts=enable_asserts)
    x = nc.declare_dram_parameter(
        "x", [P, FREE_PER_CORE], IN_MYBIR_DT, isOutput=False
    )
    y = nc.declare_dram_parameter(
        "y", [P, FREE_PER_CORE], OUT_MYBIR_DT, isOutput=True
    )
    main_f = max(tiles)
    with TileContext(nc) as tc:
        with (
            tc.tile_pool(name="in", bufs=in_bufs) as pool_in,
            tc.tile_pool(name="mid", bufs=bufs) as pool_mid,
            tc.tile_pool(name="out", bufs=bufs) as pool_out,
            tc.tile_pool(name="ins", bufs=2) as pool_in_s,
            tc.tile_pool(name="mids", bufs=2) as pool_mid_s,
            tc.tile_pool(name="outs", bufs=2) as pool_out_s,
        ):
            off = 0
            for t, f in enumerate(tiles):
                # Exact-size tiles with full-tile access patterns: sliced
                # SBUF APs measurably slow the ACT engine (~17%). Small
                # ramp/drain tiles live in their own shallow pools.
                if f == main_f:
                    pi, pm, po = pool_in, pool_mid, pool_out
                else:
                    pi, pm, po = pool_in_s, pool_mid_s, pool_out_s
                tin = pi.tile([P, f], IN_MYBIR_DT, name="i", tag=f"i{f}")
                tmid = pm.tile([P, f], MID_MYBIR_DT, name="m", tag=f"m{f}")
                tout = po.tile([P, f], OUT_MYBIR_DT, name="o", tag=f"o{f}")
                # All DMA on the sync queue: the Activation HWDGE queue was
                # measured to start servicing loads ~3us LATER than sync.
                nc.sync.dma_start(out=tin[:], in_=x[:, off : off + f])
                nc.scalar.activation(
                    tmid[:],
                    tin[:],
                    mybir.ActivationFunctionType.Tanh,
                    scale=SCALE,
                )
                nc.vector.tensor_scalar_mul(tout[:], tmid[:], OUT_SCALE)
                nc.sync.dma_start(out=y[:, off : off + f], in_=tout[:])
                off += f
    nc.compile()
    return nc


def shard_inputs(x: np.ndarray) -> list[dict[str, np.ndarray]]:
    shards = np.ascontiguousarray(x.astype(IN_NP_DT, copy=False)).reshape(
        N_CORES, P, FREE_PER_CORE
    )
    return [{"x": shards[i]} for i in range(N_CORES)]


def kernel(x: np.ndarray) -> np.ndarray:
    x = np.asarray(x)
    assert x.shape == (B, S, D), x.shape
    if "nc" not in _CACHE:
        _CACHE["nc"] = build_bass()
    nc = _CACHE["nc"]
    in_maps = shard_inputs(x)
    res = run_bass_kernel_spmd(nc, in_maps, list(range(N_CORES)))
    out = np.stack([res.results[i]["y"] for i in range(N_CORES)])
    return (out.reshape(B, S, D).astype(np.float32)) * np.float32(1.0 / OUT_SCALE)
